# revision 38
# baseline (speedup 1.0000x reference)
"""Trainium2 Bass kernel for nn_CrossEntropyLossWeight3.

Math: per row b of predict/target [B,16]:
  probs   = softmax(predict[b])
  pre     = argmax(predict[b]);  tar = argmax(target[b])
  w       = 0 if pre==tar else penalty[tar, pre]
  loss_b  = w * probs[pre]
out = mean_b(loss_b)

Key identities used on-device:
  probs[pre]   = exp(max(x)) / sum(exp(x))      (softmax at its own argmax)
  penalty[i,j] = max(c_i,c_j)/(c_i+c_j) with distinct per-class counts c;
  with u = c[pre], v = c[tar]:  w = (u != v) * max(u,v)/(u+v).
  counts/1000 (9 bits, exact) are embedded into the low mantissa bits of the
  raw inputs, so one fused embed+segmented-max DVE scan per tensor yields
  the row max together with its argmax's class count (<= 2^-14 relative
  perturbation). Two more fused custom DVE ops evaluate the whole per-row
  weight formula straight from the embedded maxima:
    WNUM = (u!=v) * max(u,v)        SPD = u + v
  so loss_b = WNUM * exp(m) / (SPD * sumexp).

v16 engine balance (per [128, 256*16] tile; single sync HWDGE ring streams
both tensors at a measured ~428 GB/s => ~9.4us/tile of DMA; DVE is the
critical engine at ~88us busy):
  - DVE     : two embed+segmax f32 scans (2 x 4.5us, hardware 1x floor for
              4-byte data) + WNUM/SPD/recip per pair + the final tile's
              bf16 sum tree (TT hits the 2x_1P perf mode: l1 = 1.2us)
  - ACT     : exp(predict) f32->bf16 (3.7us) + per-tile PSUM drains +
              exp(m); each drain is emitted AFTER the next tile's exp so
              ACT never waits on TensorE (that ping-pong cost ~10us)
  - TensorE : row sums of E for tiles 0..6 as 16 PSUM-accumulated matmuls
              each (identity weights, rhs = E[:, :, w]); strided rhs runs
              ~2.1ns/col so a tile costs ~10us of PE — fine mid-stream,
              which is why the LAST tile's sums go to the DVE tree instead
  - GPSIMD  : only small formula mults (den/num/num2); Q7 is 2-4x slower
              than DVE under DMA load and unpredictable, so it gets
              nothing latency-critical (a GPSIMD sum tree measured 13us
              while halving concurrent DVE scan throughput)
  - DMA     : both input streams + out on the SP (sync) ring so ACT's exp
              never sits in front of a dma_start issue
  - formula : per 2-tile pair, pipelined three deep: F1 (wn/sp/em/den_a/
              num) with the pair, F2a (den_b on gpsimd, after the drain)
              half a pair later, F2 (rec/num2->res) a full pair later, so
              no engine head-of-line stalls on a cross-engine dependency
  - buffers : io 3-deep, everything else 2-deep — MORE buffering is
              slower here (work bufs=3 cost 20us: SBUF port contention
              grows with concurrency); the last pair alone gets a
              dedicated e2 so exp(7) can't wait on earlier matmuls
Sharding: pure data parallel over 8 cores (batch split); each core returns
per-row partial losses [128, 2048]; host reduces and divides by B.
"""

import sys

sys.path.insert(0, "/opt/trn_rl_repo")

import numpy as np

import concourse.bass as bass
import concourse.bacc as bacc
import concourse.tile as tile
from concourse import mybir
from concourse.bass_utils import run_bass_kernel_spmd

B, W = 2097152, 16
NCORES = 8
BS = B // NCORES          # rows per core
P = 128                   # SBUF partitions
R = 256                   # rows per partition per tile
F = R * W                 # free elems per partition per tile
TILE_ROWS = P * R
NT = BS // TILE_ROWS      # tiles per core

LABELS_NUM_COUNT = [500000, 120000, 80000, 45000, 30000, 250000, 15000, 9000,
                    60000, 7000, 180000, 22000, 11000, 95000, 5000, 40000]

f32 = mybir.dt.float32
bf16 = mybir.dt.bfloat16
u32 = mybir.dt.uint32
AX = mybir.AxisListType
OP = mybir.AluOpType
ACT = mybir.ActivationFunctionType

PAYLOAD_BITS = 9          # counts/1000 <= 500 fits in 9 bits exactly
PAYLOAD_MASK = (1 << PAYLOAD_BITS) - 1
F_2P23 = 8388608.0        # bit pattern 0x4B000000; OR'ing these bits onto the
                          # 9-bit payload makes the exact float 2^23 + payload
F_2P24 = 16777216.0


def _register_custom_ops():
    """Three runtime-registered custom DVE ops.

    EMBMAX_SEG_ANT: fused "embed payload + segmented max" scan (see v2/v3
      history): body = Scan(MAX, ((x|c)^c)|pay, _subdim_step=Zero) over a
      [P, S, 16] view; stride-0 out leaves per-segment maxima in [P, S].
      The OR/XOR form avoids an AND with 0xFFFFFE00 (NaN bit pattern).
    WNUM_ANT(me, mt; s0=mask, s1=2^23): with u' = (me & mask) | bits(s1),
      v' = (mt & mask) | bits(s1)  (both exact floats 2^23 + count):
      out = (u' != v') * (max(u',v') - 2^23) = (u!=v)*max(u,v).
    SPD_ANT(me, mt; s0, s1, imm2=2^24): out = u' + v' - 2^24 = u + v.
    """
    import numpy as np_

    from concourse.dve_spec import (
        Spec, Src0, Src1, C0, C1, C2, Bin, AluOp, lower, ne, maxx, Zero,
    )
    from concourse.dve_ops import (
        DveOp,
        OPS,
        CUSTOM_DVE_SPECS,
        _SUB_OPCODE_FOR_NAME,
        _CUSTOM_DVE_ROW_BASE,
        _COMPILE_CACHE,
    )
    from concourse.dve_uop import DveOpSpec
    import concourse.dve_spec as ds

    def reg(name, spec, rd1):
        for o in OPS:
            if o.name == name:
                return o
        shas = {}
        for ver in ("v3", "v4"):
            uops = lower(spec, ver=ver)
            s = DveOpSpec(
                name=name,
                opcode=_CUSTOM_DVE_ROW_BASE + len(OPS),
                uops=uops,
                rd1_en=rd1,
            )
            shas[ver] = s.sha(ver)
        op = DveOp(name, spec, subdim=False, uops_sha=shas)
        _SUB_OPCODE_FOR_NAME[name] = _CUSTOM_DVE_ROW_BASE + len(OPS)
        OPS.append(op)
        CUSTOM_DVE_SPECS[name] = spec
        return op

    embed_expr = Bin(
        AluOp.BITWISE_OR,
        Bin(AluOp.BITWISE_XOR, Bin(AluOp.BITWISE_OR, Src0, C0), C0),
        Src1,
    )

    def _ref_embmax(in0, in1, s0, s1, imm2):
        emb = (
            ((in0.view(np_.uint32) | PAYLOAD_MASK) ^ PAYLOAD_MASK)
            | in1.view(np_.uint32)
        ).view(np_.float32)
        return np_.maximum.accumulate(emb, axis=-1)

    def reg_embmax():
        name = "EMBMAX_SEG_ANT"
        for o in OPS:
            if o.name == name:
                return o
        seg = ds.Scan(op=AluOp.MAX, expr=embed_expr, init=None, _subdim_step=Zero)
        spec = Spec(body=seg, reference=_ref_embmax)
        orig_so, orig_nas = ds._scan_overrides, ds._node_as_stage

        def patched_so(scans, node_stage):
            seed, step = {}, {}
            for scan in scans:
                d = node_stage[scan]
                init = (
                    scan.init
                    if scan.init is not None
                    else ds._ACCUM_IDENTITY[scan.op]
                )
                seed[d] = orig_nas(init)
                if scan._subdim_step is not None:
                    step[d] = ds._Stage(AluOp.BYPASS, scan.expr)
            return seed, step

        def patched_nas(e):
            if isinstance(e, ds.Scan) and e._subdim_step is not None:
                return ds._Stage(e.op, ds.AluInp.CURR_ALU_OUT, e.expr)
            return orig_nas(e)

        uops_by_ver, shas = {}, {}
        ds._scan_overrides, ds._node_as_stage = patched_so, patched_nas
        try:
            for ver in ("v3", "v4"):
                uops_by_ver[ver] = lower(spec, ver=ver)
        finally:
            ds._scan_overrides, ds._node_as_stage = orig_so, orig_nas
        opcode = _CUSTOM_DVE_ROW_BASE + len(OPS)
        for ver in ("v3", "v4"):
            s = DveOpSpec(name=name, opcode=opcode, uops=uops_by_ver[ver], rd1_en=True)
            shas[ver] = s.sha(ver)
            _COMPILE_CACHE[(name, ver)] = s
        op = DveOp(name, spec, subdim=True, uops_sha=shas)
        _SUB_OPCODE_FOR_NAME[name] = opcode
        OPS.append(op)
        CUSTOM_DVE_SPECS[name] = spec
        return op

    def _uprime(src):
        return Bin(AluOp.BITWISE_OR, Bin(AluOp.BITWISE_AND, src, C0), C1)

    def _np_uprime(x):
        return (
            (x.view(np_.uint32) & PAYLOAD_MASK) | np_.uint32(0x4B000000)
        ).view(np_.float32)

    up_e, vp_e = _uprime(Src0), _uprime(Src1)
    wnum_spec = Spec(
        body=Bin(
            AluOp.MULTIPLY,
            ne(up_e, vp_e),
            Bin(AluOp.SUBTRACT, maxx(up_e, vp_e), C1),
        ),
        reference=lambda in0, in1, s0, s1, imm2: np_.where(
            _np_uprime(in0) != _np_uprime(in1),
            np_.maximum(_np_uprime(in0), _np_uprime(in1)) - np_.float32(F_2P23),
            np_.float32(0.0),
        ).astype(np_.float32),
    )
    spd_spec = Spec(
        body=Bin(
            AluOp.SUBTRACT, Bin(AluOp.ADD, up_e, vp_e), C2
        ),
        reference=lambda in0, in1, s0, s1, imm2: (
            _np_uprime(in0) + _np_uprime(in1) - np_.float32(F_2P24)
        ).astype(np_.float32),
    )

    embed = reg_embmax()
    wnum = reg("WNUM_ANT", wnum_spec, rd1=True)
    spd = reg("SPD_ANT", spd_spec, rd1=True)
    return embed, wnum, spd


BW = 2 * R                 # max formula block width (one full-size pair)
PAIR_R = [256, 256, 256, 256]        # rows/partition per tile, by pair
                           # (tried [128,256,256,256,128] to start scans
                           # earlier and shrink the tail: 10 tiles raised
                           # every engine's busy 15-20%% from op count +
                           # concurrency pressure — uniform 8 tiles wins)
RES_COLS = 2 * sum(PAIR_R)           # per-row loss columns (= BS // P)


def _emit_tile(nc, pools, xp_v, xt_v, pay_b, rt, sub, embed_op, mask_ap,
               me, mt, e2):
    """Streaming part for one [128, rt*16] tile. Row stats land in column
    half `sub` of the pair stats tiles me/mt; exp lands in half `sub` of
    the bf16 pair tile e2."""
    io_pool = pools[0]
    ft = rt * W
    cols = slice(sub * rt, (sub + 1) * rt)

    # both input streams on the sync HWDGE ring: SP issues nothing else, so
    # dma_starts go out back-to-back and are never stuck behind an ACT op
    xp = io_pool.tile([P, F], f32, tag="xp")
    nc.sync.dma_start(out=xp[:, :ft], in_=xp_v)
    xt = io_pool.tile([P, F], f32, tag="xt")
    nc.sync.dma_start(out=xt[:, :ft], in_=xt_v)

    # fused embed + segmented max over RAW predict on DVE; runs concurrently
    # with the exp pass on ACT (both only read xp)
    xp3 = xp[:, :ft].rearrange("p (r w) -> p r w", w=W)
    nc.vector._custom_dve(
        embed_op,
        out=me[:, cols].unsqueeze(2).broadcast_to([P, rt, W]),
        in0=xp3, in1=pay_b[:, :rt, :], s0=mask_ap,
    )

    # E = exp(predict) on ScalarE into half `sub` of the bf16 pair tile
    # (contiguous write — a transposed write ran 5x slower on ACT)
    nc.scalar.activation(e2[:, sub * ft:(sub + 1) * ft], xp[:, :ft],
                         ACT.Exp)

    # target side: fused embed + segmented max on DVE
    xt3 = xt[:, :ft].rearrange("p (r w) -> p r w", w=W)
    nc.vector._custom_dve(
        embed_op,
        out=mt[:, cols].unsqueeze(2).broadcast_to([P, rt, W]),
        in0=xt3, in1=pay_b[:, :rt, :], s0=mask_ap,
    )


def _emit_sums_pe(nc, pools, e2, rt, sub, ident_b):
    """Row sums of one e2 half (one tile) on TensorE: 16 matmuls with
    identity weights, one per class column (strided rhs -> ~2.1ns/col on
    HW), PSUM-accumulated in f32. Issued per tile (right after its exp) so
    PE work spreads across the whole stream. Returns the PSUM tile; the
    ACT drain to SBUF is emitted by the caller one tile LATER, so the next
    exp in ACT program order never waits on this tile's matmuls."""
    ps_pool = pools[4]
    ft = rt * W
    s2 = ps_pool.tile([P, R], f32, tag="s2")
    e3h = e2[:, sub * ft:(sub + 1) * ft].rearrange("p (r w) -> p r w", w=W)
    for w in range(W):
        nc.tensor.matmul(
            out=s2[:, :rt], lhsT=ident_b[:, :], rhs=e3h[:, :, w],
            start=(w == 0), stop=(w == W - 1),
        )
    return s2


def _emit_sums_dve_tree(nc, pools, e2, rt, sub, s2c):
    """Row sums of one e2 half via the contiguous-halves bf16 pairwise-add
    tree on DVE (TT hits the 2x_1P perf mode). Used ONLY for the final
    tile: the PE can't start that tile's matmuls until the stream has
    already ended, and the GPSIMD tree there ran ~13us while halving
    concurrent DVE scan speed."""
    work_pool = pools[1]
    ft = rt * W
    e3 = e2[:, sub * ft:(sub + 1) * ft].rearrange("p (r w) -> p r w", w=W)
    l1 = work_pool.tile([P, R * 8], bf16, tag="l1")
    l1v = l1[:, :rt * 8].rearrange("p (r h) -> p r h", h=8)
    nc.vector.tensor_tensor(l1v, e3[:, :, 0:8], e3[:, :, 8:16], op=OP.add)
    l2 = work_pool.tile([P, R * 4], bf16, tag="l2")
    l2v = l2[:, :rt * 4].rearrange("p (r h) -> p r h", h=4)
    nc.vector.tensor_tensor(l2v, l1v[:, :, 0:4], l1v[:, :, 4:8], op=OP.add)
    l3 = work_pool.tile([P, R * 2], bf16, tag="l3")
    l3v = l3[:, :rt * 2].rearrange("p (r h) -> p r h", h=2)
    nc.vector.tensor_tensor(l3v, l2v[:, :, 0:2], l2v[:, :, 2:4], op=OP.add)
    nc.vector.tensor_tensor(
        s2c[:, sub * rt:(sub + 1) * rt].unsqueeze(2),
        l3v[:, :, 0:1], l3v[:, :, 1:2], op=OP.add)


def _emit_f1(nc, pools, me, mt, s2c, rt, ops, mask_ap, last):
    """Formula stage 1 for one [128, 2*rt] pair block (emitted with its
    odd tile): everything that only needs me/mt/s.
      wn = (u!=v)*max(u,v)   sp = u+v   em = exp(m)
      den = sp * sumexp      num = wn * em
    den/num run on GPSIMD (near idle) except for the last block, where
    DVE's ops shorten the post-DMA tail."""
    fp_pool = pools[3]
    _, wnum_op, spd_op = ops
    bw = 2 * rt
    mul_eng = nc.vector if last else nc.gpsimd

    wn = fp_pool.tile([P, BW], f32, tag="wn")
    nc.vector._custom_dve(wnum_op, out=wn[:, :bw], in0=me[:, :bw],
                          in1=mt[:, :bw], s0=mask_ap, s1=F_2P23)
    sp = fp_pool.tile([P, BW], f32, tag="sp")
    nc.vector._custom_dve(spd_op, out=sp[:, :bw], in0=me[:, :bw],
                          in1=mt[:, :bw], s0=mask_ap, s1=F_2P23,
                          imm2=F_2P24)
    # em = exp(m): payload bits perturb m by <= 2^-14 relative — in budget
    em = fp_pool.tile([P, BW], f32, tag="em")
    nc.scalar.activation(em[:, :bw], me[:, :bw], ACT.Exp)

    den = fp_pool.tile([P, BW], f32, tag="dn")
    if last:
        # both s2c halves are already written (tile NT-2 drained during
        # tile NT-1; tile NT-1 via the DVE tree): one full-width den
        mul_eng.tensor_tensor(den[:, :bw], sp[:, :bw], s2c[:, :bw],
                              op=OP.mult)
    else:
        # s2c's second half is only drained from PSUM during the NEXT
        # tile (deferred ACT drain), so den's second half moves to F2a
        mul_eng.tensor_tensor(den[:, :rt], sp[:, :rt], s2c[:, :rt],
                              op=OP.mult)
    num = fp_pool.tile([P, BW], f32, tag="nm")
    mul_eng.tensor_tensor(num[:, :bw], wn[:, :bw], em[:, :bw], op=OP.mult)
    return sp, s2c, den, num, rt


def _emit_f2a(nc, sp, s2c, den, rt):
    """Finish den's second half on GPSIMD (emitted at the next pair's
    first tile, right after the ACT drain that writes s2c's second half,
    and a full half-pair before DVE's rec needs it — no head-of-line
    stall)."""
    nc.gpsimd.tensor_tensor(den[:, rt:2 * rt], sp[:, rt:2 * rt],
                            s2c[:, rt:2 * rt], op=OP.mult)


def _emit_f2(nc, pools, res_sl, sp, s2c, den, num, rt, last):
    """Formula stage 2 (emitted 1.5 tiles later so no engine head-of-line
    stalls on a cross-engine dep): res = num / den."""
    fp_pool = pools[3]
    bw = 2 * rt
    rec = fp_pool.tile([P, BW], f32, tag="rc")
    nc.vector.reciprocal_approx_fast(out=rec[:, :bw], in_=den[:, :bw])
    mul_eng = nc.vector if last else nc.gpsimd
    mul_eng.tensor_tensor(res_sl, num[:, :bw], rec[:, :bw], op=OP.mult)


def _emit_pass(nc, pools, pred, targ, pay_b, ident_b, res, ops, mask_ap):
    _, work_pool, stats_pool, fp_pool, ps_pool, lastp_pool = pools
    embed_op = ops[0]
    pend = None            # F1 outputs of the previous pair block
    pend_drain = None      # (psum_tile, s2c, half, rt) not yet drained
    base = 0               # DRAM row offset of the next tile
    roff = 0               # res column offset of the current pair
    ntiles = 2 * len(PAIR_R)
    ti = 0                 # flat tile counter
    for k, rt in enumerate(PAIR_R):
        bw = 2 * rt
        me = stats_pool.tile([P, BW], f32, tag="me")
        mt = stats_pool.tile([P, BW], f32, tag="mt")
        s2c = fp_pool.tile([P, BW], bf16, tag="s2c")
        # dedicated e2 for the last pair: its exp must not wait for an
        # earlier pair's matmuls to release the shared 2-buf rotation
        if k == len(PAIR_R) - 1:
            e2 = lastp_pool.tile([P, 2 * F], bf16, tag="e2last")
        else:
            e2 = work_pool.tile([P, 2 * F], bf16, tag="e2")
        for sub in range(2):
            nrows = P * rt
            xp_v = pred[base:base + nrows, :].rearrange(
                "(p r) w -> p (r w)", p=P, r=rt)
            xt_v = targ[base:base + nrows, :].rearrange(
                "(p r) w -> p (r w)", p=P, r=rt)
            base += nrows
            _emit_tile(nc, pools, xp_v, xt_v, pay_b, rt, sub, embed_op,
                       mask_ap, me, mt, e2)
            # drain the PREVIOUS tile's PSUM sums now — after this tile's
            # exp in ACT program order, so exp never waits on matmuls
            if pend_drain is not None:
                ps, dst, dhalf, drt = pend_drain
                nc.scalar.activation(dst[:, dhalf * drt:(dhalf + 1) * drt],
                                     ps[:, :drt], ACT.Copy)
                pend_drain = None
            if ti < ntiles - 1:
                pend_drain = (_emit_sums_pe(nc, pools, e2, rt, sub,
                                            ident_b), s2c, sub, rt)
            else:
                _emit_sums_dve_tree(nc, pools, e2, rt, sub, s2c)
            # interleave the previous pair's F2 across this pair
            if pend is not None and sub == 0:
                _emit_f2a(nc, pend[0], pend[1], pend[2], pend[4])
            if pend is not None and sub == 1:
                pbw = 2 * pend[4]
                _emit_f2(nc, pools, res[:, roff - pbw:roff], *pend,
                         last=False)
                pend = None
            ti += 1
        roff += bw
        pend = _emit_f1(nc, pools, me, mt, s2c, rt, ops, mask_ap,
                        last=(k == len(PAIR_R) - 1))
    _emit_f2(nc, pools, res[:, roff - 2 * pend[4]:roff], *pend, last=True)


def _build_program():
    nc = bacc.Bacc("TRN2", target_bir_lowering=False, debug=False)
    pred = nc.dram_tensor("predict", [BS, W], f32, kind="ExternalInput")
    targ = nc.dram_tensor("target", [BS, W], f32, kind="ExternalInput")
    pay = nc.dram_tensor("payload", [P, W], u32, kind="ExternalInput")
    ident = nc.dram_tensor("ident", [P, P], f32, kind="ExternalInput")
    out = nc.dram_tensor("out", [P, RES_COLS], f32, kind="ExternalOutput")

    with tile.TileContext(nc) as tc:
        with (
            tc.tile_pool(name="io", bufs=2) as io_pool,
            tc.tile_pool(name="work", bufs=2) as work_pool,
            tc.tile_pool(name="stats", bufs=2) as stats_pool,
            tc.tile_pool(name="fp", bufs=2) as fp_pool,
            tc.psum_pool(name="ps", bufs=3) as ps_pool,
            tc.tile_pool(name="lastp", bufs=1) as lastp_pool,
            tc.tile_pool(name="const", bufs=1) as const_pool,
        ):
            pay_t = const_pool.tile([P, W], u32, tag="pay")
            nc.gpsimd.dma_start(out=pay_t[:, :], in_=pay[:, :])
            pay_b = pay_t[:, :].unsqueeze(1).broadcast_to([P, R, W]).bitcast(f32)

            ident_t = const_pool.tile([P, P], f32, tag="idf")
            nc.gpsimd.dma_start(out=ident_t[:, :], in_=ident[:, :])
            ident_b = const_pool.tile([P, P], bf16, tag="idb")
            nc.scalar.activation(ident_b[:, :], ident_t[:, :], ACT.Copy)

            mask_t = const_pool.tile([P, 1], u32, tag="mask")
            nc.vector.memset(mask_t[:, :], PAYLOAD_MASK)
            mask_ap = mask_t[:, :1].bitcast(f32)

            res = const_pool.tile([P, RES_COLS], f32, tag="res")

            ops = _register_custom_ops()
            pools = (io_pool, work_pool, stats_pool, fp_pool, ps_pool, lastp_pool)
            _emit_pass(nc, pools, pred[:, :], targ[:, :], pay_b, ident_b,
                       res, ops, mask_ap)

            nc.sync.dma_start(out=out[:, :], in_=res[:, :])
    nc.compile()
    return nc


_CACHE = {}


def _run(predict, target, trace=False):
    if "nc" not in _CACHE:
        _CACHE["nc"] = _build_program()
    nc = _CACHE["nc"]

    predict = np.ascontiguousarray(np.asarray(predict, dtype=np.float32))
    target = np.ascontiguousarray(np.asarray(target, dtype=np.float32))
    payload = np.broadcast_to(
        (np.asarray(LABELS_NUM_COUNT, dtype=np.uint32) // 1000)[None, :], (P, W)
    ).copy()
    ident = np.eye(P, dtype=np.float32)

    in_maps = []
    for i in range(NCORES):
        in_maps.append(
            {
                "predict": predict[i * BS : (i + 1) * BS],
                "target": target[i * BS : (i + 1) * BS],
                "payload": payload,
                "ident": ident,
            }
        )
    res = run_bass_kernel_spmd(nc, in_maps, core_ids=list(range(NCORES)), trace=trace)
    total = np.float64(0.0)
    for r in res.results:
        total += np.float64(r["out"].astype(np.float64).sum())
    value = np.float32(total / B)
    return np.asarray(value, dtype=np.float32), res


def kernel(predict, target, penalty_matrix=None):
    value, _ = _run(predict, target, trace=False)
    return value



# revision 39
# speedup vs baseline: 1.0628x; 1.0628x over previous
"""Trainium2 Bass kernel for nn_CrossEntropyLossWeight3.

Math: per row b of predict/target [B,16]:
  probs   = softmax(predict[b])
  pre     = argmax(predict[b]);  tar = argmax(target[b])
  w       = 0 if pre==tar else penalty[tar, pre]
  loss_b  = w * probs[pre]
out = mean_b(loss_b)

Key identities used on-device:
  probs[pre]   = exp(max(x)) / sum(exp(x))      (softmax at its own argmax)
  penalty[i,j] = max(c_i,c_j)/(c_i+c_j) with distinct per-class counts c;
  with u = c[pre], v = c[tar]:  w = (u != v) * max(u,v)/(u+v).
  counts/1000 (9 bits, exact) are embedded into the low mantissa bits of the
  raw inputs, so one fused embed+segmented-max DVE scan per tensor yields
  the row max together with its argmax's class count (<= 2^-14 relative
  perturbation). Two more fused custom DVE ops evaluate the whole per-row
  weight formula straight from the embedded maxima:
    WNUM = (u!=v) * max(u,v)        SPD = u + v
  so loss_b = WNUM * exp(m) / (SPD * sumexp).

v16 engine balance (per [128, 256*16] tile; single sync HWDGE ring streams
both tensors at a measured ~428 GB/s => ~9.4us/tile of DMA; DVE is the
critical engine at ~88us busy):
  - DVE     : two embed+segmax f32 scans (2 x 4.5us, hardware 1x floor for
              4-byte data) + WNUM/SPD/recip per pair + the final tile's
              bf16 sum tree (TT hits the 2x_1P perf mode: l1 = 1.2us)
  - ACT     : exp(predict) f32->bf16 (3.7us) + per-tile PSUM drains +
              exp(m); each drain is emitted AFTER the next tile's exp so
              ACT never waits on TensorE (that ping-pong cost ~10us)
  - TensorE : row sums of E for tiles 0..6 as 16 PSUM-accumulated matmuls
              each (identity weights, rhs = E[:, :, w]); strided rhs runs
              ~2.1ns/col so a tile costs ~10us of PE — fine mid-stream,
              which is why the LAST tile's sums go to the DVE tree instead
  - GPSIMD  : only small formula mults (den/num/num2); Q7 is 2-4x slower
              than DVE under DMA load and unpredictable, so it gets
              nothing latency-critical (a GPSIMD sum tree measured 13us
              while halving concurrent DVE scan throughput)
  - DMA     : both input streams + out on the SP (sync) ring so ACT's exp
              never sits in front of a dma_start issue
  - formula : per 2-tile pair, pipelined three deep: F1 (wn/sp/em/den_a/
              num) with the pair, F2a (den_b on gpsimd, after the drain)
              half a pair later, F2 (rec/num2->res) a full pair later, so
              no engine head-of-line stalls on a cross-engine dependency
  - buffers : io 3-deep, everything else 2-deep — MORE buffering is
              slower here (work bufs=3 cost 20us: SBUF port contention
              grows with concurrency); the last pair alone gets a
              dedicated e2 so exp(7) can't wait on earlier matmuls
Sharding: pure data parallel over 8 cores (batch split); each core returns
per-row partial losses [128, 2048]; host reduces and divides by B.
"""

import sys

sys.path.insert(0, "/opt/trn_rl_repo")

import numpy as np

import concourse.bass as bass
import concourse.bacc as bacc
import concourse.tile as tile
from concourse import mybir
from concourse.bass_utils import run_bass_kernel_spmd

B, W = 2097152, 16
NCORES = 8
BS = B // NCORES          # rows per core
P = 128                   # SBUF partitions
R = 256                   # rows per partition per tile
F = R * W                 # free elems per partition per tile
TILE_ROWS = P * R
NT = BS // TILE_ROWS      # tiles per core

LABELS_NUM_COUNT = [500000, 120000, 80000, 45000, 30000, 250000, 15000, 9000,
                    60000, 7000, 180000, 22000, 11000, 95000, 5000, 40000]

f32 = mybir.dt.float32
bf16 = mybir.dt.bfloat16
u32 = mybir.dt.uint32
AX = mybir.AxisListType
OP = mybir.AluOpType
ACT = mybir.ActivationFunctionType

PAYLOAD_BITS = 9          # counts/1000 <= 500 fits in 9 bits exactly
PAYLOAD_MASK = (1 << PAYLOAD_BITS) - 1
F_2P23 = 8388608.0        # bit pattern 0x4B000000; OR'ing these bits onto the
                          # 9-bit payload makes the exact float 2^23 + payload
F_2P24 = 16777216.0


def _register_custom_ops():
    """Three runtime-registered custom DVE ops.

    EMBMAX_SEG_ANT: fused "embed payload + segmented max" scan (see v2/v3
      history): body = Scan(MAX, ((x|c)^c)|pay, _subdim_step=Zero) over a
      [P, S, 16] view; stride-0 out leaves per-segment maxima in [P, S].
      The OR/XOR form avoids an AND with 0xFFFFFE00 (NaN bit pattern).
    WNUM_ANT(me, mt; s0=mask, s1=2^23): with u' = (me & mask) | bits(s1),
      v' = (mt & mask) | bits(s1)  (both exact floats 2^23 + count):
      out = (u' != v') * (max(u',v') - 2^23) = (u!=v)*max(u,v).
    SPD_ANT(me, mt; s0, s1, imm2=2^24): out = u' + v' - 2^24 = u + v.
    """
    import numpy as np_

    from concourse.dve_spec import (
        Spec, Src0, Src1, C0, C1, C2, Bin, AluOp, lower, ne, maxx, Zero,
    )
    from concourse.dve_ops import (
        DveOp,
        OPS,
        CUSTOM_DVE_SPECS,
        _SUB_OPCODE_FOR_NAME,
        _CUSTOM_DVE_ROW_BASE,
        _COMPILE_CACHE,
    )
    from concourse.dve_uop import DveOpSpec
    import concourse.dve_spec as ds

    def reg(name, spec, rd1):
        for o in OPS:
            if o.name == name:
                return o
        shas = {}
        for ver in ("v3", "v4"):
            uops = lower(spec, ver=ver)
            s = DveOpSpec(
                name=name,
                opcode=_CUSTOM_DVE_ROW_BASE + len(OPS),
                uops=uops,
                rd1_en=rd1,
            )
            shas[ver] = s.sha(ver)
        op = DveOp(name, spec, subdim=False, uops_sha=shas)
        _SUB_OPCODE_FOR_NAME[name] = _CUSTOM_DVE_ROW_BASE + len(OPS)
        OPS.append(op)
        CUSTOM_DVE_SPECS[name] = spec
        return op

    embed_expr = Bin(
        AluOp.BITWISE_OR,
        Bin(AluOp.BITWISE_XOR, Bin(AluOp.BITWISE_OR, Src0, C0), C0),
        Src1,
    )

    def _ref_embmax(in0, in1, s0, s1, imm2):
        emb = (
            ((in0.view(np_.uint32) | PAYLOAD_MASK) ^ PAYLOAD_MASK)
            | in1.view(np_.uint32)
        ).view(np_.float32)
        return np_.maximum.accumulate(emb, axis=-1)

    def reg_embmax():
        name = "EMBMAX_SEG_ANT"
        for o in OPS:
            if o.name == name:
                return o
        seg = ds.Scan(op=AluOp.MAX, expr=embed_expr, init=None, _subdim_step=Zero)
        spec = Spec(body=seg, reference=_ref_embmax)
        orig_so, orig_nas = ds._scan_overrides, ds._node_as_stage

        def patched_so(scans, node_stage):
            seed, step = {}, {}
            for scan in scans:
                d = node_stage[scan]
                init = (
                    scan.init
                    if scan.init is not None
                    else ds._ACCUM_IDENTITY[scan.op]
                )
                seed[d] = orig_nas(init)
                if scan._subdim_step is not None:
                    step[d] = ds._Stage(AluOp.BYPASS, scan.expr)
            return seed, step

        def patched_nas(e):
            if isinstance(e, ds.Scan) and e._subdim_step is not None:
                return ds._Stage(e.op, ds.AluInp.CURR_ALU_OUT, e.expr)
            return orig_nas(e)

        uops_by_ver, shas = {}, {}
        ds._scan_overrides, ds._node_as_stage = patched_so, patched_nas
        try:
            for ver in ("v3", "v4"):
                uops_by_ver[ver] = lower(spec, ver=ver)
        finally:
            ds._scan_overrides, ds._node_as_stage = orig_so, orig_nas
        opcode = _CUSTOM_DVE_ROW_BASE + len(OPS)
        for ver in ("v3", "v4"):
            s = DveOpSpec(name=name, opcode=opcode, uops=uops_by_ver[ver], rd1_en=True)
            shas[ver] = s.sha(ver)
            _COMPILE_CACHE[(name, ver)] = s
        op = DveOp(name, spec, subdim=True, uops_sha=shas)
        _SUB_OPCODE_FOR_NAME[name] = opcode
        OPS.append(op)
        CUSTOM_DVE_SPECS[name] = spec
        return op

    def _uprime(src):
        return Bin(AluOp.BITWISE_OR, Bin(AluOp.BITWISE_AND, src, C0), C1)

    def _np_uprime(x):
        return (
            (x.view(np_.uint32) & PAYLOAD_MASK) | np_.uint32(0x4B000000)
        ).view(np_.float32)

    up_e, vp_e = _uprime(Src0), _uprime(Src1)
    wnum_spec = Spec(
        body=Bin(
            AluOp.MULTIPLY,
            ne(up_e, vp_e),
            Bin(AluOp.SUBTRACT, maxx(up_e, vp_e), C1),
        ),
        reference=lambda in0, in1, s0, s1, imm2: np_.where(
            _np_uprime(in0) != _np_uprime(in1),
            np_.maximum(_np_uprime(in0), _np_uprime(in1)) - np_.float32(F_2P23),
            np_.float32(0.0),
        ).astype(np_.float32),
    )
    spd_spec = Spec(
        body=Bin(
            AluOp.SUBTRACT, Bin(AluOp.ADD, up_e, vp_e), C2
        ),
        reference=lambda in0, in1, s0, s1, imm2: (
            _np_uprime(in0) + _np_uprime(in1) - np_.float32(F_2P24)
        ).astype(np_.float32),
    )

    embed = reg_embmax()
    wnum = reg("WNUM_ANT", wnum_spec, rd1=True)
    spd = reg("SPD_ANT", spd_spec, rd1=True)
    return embed, wnum, spd


BW = 2 * R                 # max formula block width (one full-size pair)
PAIR_R = [256, 256, 256, 256]        # rows/partition per tile, by pair
                           # (tried [128,256,256,256,128] to start scans
                           # earlier and shrink the tail: 10 tiles raised
                           # every engine's busy 15-20%% from op count +
                           # concurrency pressure — uniform 8 tiles wins)
RES_COLS = 2 * sum(PAIR_R)           # per-row loss columns (= BS // P)


def _emit_tile(nc, pools, xp_v, xt_v, pay_b, rt, sub, embed_op, mask_ap,
               me, mt, e2):
    """Streaming part for one [128, rt*16] tile. Row stats land in column
    half `sub` of the pair stats tiles me/mt; exp lands in half `sub` of
    the bf16 pair tile e2."""
    io_pool = pools[0]
    ft = rt * W
    cols = slice(sub * rt, (sub + 1) * rt)

    # both input streams on the sync HWDGE ring: SP issues nothing else, so
    # dma_starts go out back-to-back and are never stuck behind an ACT op
    xp = io_pool.tile([P, F], f32, tag="xp")
    nc.sync.dma_start(out=xp[:, :ft], in_=xp_v)
    xt = io_pool.tile([P, F], f32, tag="xt")
    nc.sync.dma_start(out=xt[:, :ft], in_=xt_v)

    # fused embed + segmented max over RAW predict on DVE; runs concurrently
    # with the exp pass on ACT (both only read xp)
    xp3 = xp[:, :ft].rearrange("p (r w) -> p r w", w=W)
    nc.vector._custom_dve(
        embed_op,
        out=me[:, cols].unsqueeze(2).broadcast_to([P, rt, W]),
        in0=xp3, in1=pay_b[:, :rt, :], s0=mask_ap,
    )

    # E = exp(predict) on ScalarE into half `sub` of the bf16 pair tile
    # (contiguous write — a transposed write ran 5x slower on ACT)
    nc.scalar.activation(e2[:, sub * ft:(sub + 1) * ft], xp[:, :ft],
                         ACT.Exp)

    # target side: fused embed + segmented max on DVE
    xt3 = xt[:, :ft].rearrange("p (r w) -> p r w", w=W)
    nc.vector._custom_dve(
        embed_op,
        out=mt[:, cols].unsqueeze(2).broadcast_to([P, rt, W]),
        in0=xt3, in1=pay_b[:, :rt, :], s0=mask_ap,
    )


def _emit_sums_pe(nc, pools, e2, rt, sub, ident_b):
    """Row sums of one e2 half (one tile) on TensorE: 16 matmuls with
    identity weights, one per class column (strided rhs -> ~2.1ns/col on
    HW), PSUM-accumulated in f32. Issued per tile (right after its exp) so
    PE work spreads across the whole stream. Returns the PSUM tile; the
    ACT drain to SBUF is emitted by the caller one tile LATER, so the next
    exp in ACT program order never waits on this tile's matmuls."""
    ps_pool = pools[4]
    ft = rt * W
    s2 = ps_pool.tile([P, R], f32, tag="s2")
    e3h = e2[:, sub * ft:(sub + 1) * ft].rearrange("p (r w) -> p r w", w=W)
    for w in range(W):
        nc.tensor.matmul(
            out=s2[:, :rt], lhsT=ident_b[:, :], rhs=e3h[:, :, w],
            start=(w == 0), stop=(w == W - 1),
        )
    return s2


def _emit_sums_dve_tree(nc, pools, e2, rt, sub, s2c):
    """Row sums of one e2 half via the contiguous-halves bf16 pairwise-add
    tree on DVE (TT hits the 2x_1P perf mode). Used ONLY for the final
    tile: the PE can't start that tile's matmuls until the stream has
    already ended, and the GPSIMD tree there ran ~13us while halving
    concurrent DVE scan speed."""
    work_pool = pools[1]
    ft = rt * W
    e3 = e2[:, sub * ft:(sub + 1) * ft].rearrange("p (r w) -> p r w", w=W)
    l1 = work_pool.tile([P, R * 8], bf16, tag="l1")
    l1v = l1[:, :rt * 8].rearrange("p (r h) -> p r h", h=8)
    nc.vector.tensor_tensor(l1v, e3[:, :, 0:8], e3[:, :, 8:16], op=OP.add)
    l2 = work_pool.tile([P, R * 4], bf16, tag="l2")
    l2v = l2[:, :rt * 4].rearrange("p (r h) -> p r h", h=4)
    nc.vector.tensor_tensor(l2v, l1v[:, :, 0:4], l1v[:, :, 4:8], op=OP.add)
    l3 = work_pool.tile([P, R * 2], bf16, tag="l3")
    l3v = l3[:, :rt * 2].rearrange("p (r h) -> p r h", h=2)
    nc.vector.tensor_tensor(l3v, l2v[:, :, 0:2], l2v[:, :, 2:4], op=OP.add)
    nc.vector.tensor_tensor(
        s2c[:, sub * rt:(sub + 1) * rt].unsqueeze(2),
        l3v[:, :, 0:1], l3v[:, :, 1:2], op=OP.add)


def _emit_f1(nc, pools, me, mt, s2c, rt, ops, mask_ap, last):
    """Formula stage 1 for one [128, 2*rt] pair block (emitted with its
    odd tile): everything that only needs me/mt/s.
      wn = (u!=v)*max(u,v)   sp = u+v   em = exp(m)
      den = sp * sumexp      num = wn * em
    den/num run on GPSIMD (near idle) except for the last block, where
    DVE's ops shorten the post-DMA tail."""
    fp_pool = pools[3]
    _, wnum_op, spd_op = ops
    bw = 2 * rt
    mul_eng = nc.vector if last else nc.gpsimd

    wn = fp_pool.tile([P, BW], f32, tag="wn")
    nc.vector._custom_dve(wnum_op, out=wn[:, :bw], in0=me[:, :bw],
                          in1=mt[:, :bw], s0=mask_ap, s1=F_2P23)
    sp = fp_pool.tile([P, BW], f32, tag="sp")
    nc.vector._custom_dve(spd_op, out=sp[:, :bw], in0=me[:, :bw],
                          in1=mt[:, :bw], s0=mask_ap, s1=F_2P23,
                          imm2=F_2P24)
    # em = exp(m): payload bits perturb m by <= 2^-14 relative — in budget
    em = fp_pool.tile([P, BW], f32, tag="em")
    nc.scalar.activation(em[:, :bw], me[:, :bw], ACT.Exp)

    den = fp_pool.tile([P, BW], f32, tag="dn")
    if last:
        # both s2c halves are already written (tile NT-2 drained during
        # tile NT-1; tile NT-1 via the DVE tree): one full-width den
        mul_eng.tensor_tensor(den[:, :bw], sp[:, :bw], s2c[:, :bw],
                              op=OP.mult)
    else:
        # s2c's second half is only drained from PSUM during the NEXT
        # tile (deferred ACT drain), so den's second half moves to F2a
        mul_eng.tensor_tensor(den[:, :rt], sp[:, :rt], s2c[:, :rt],
                              op=OP.mult)
    num = fp_pool.tile([P, BW], f32, tag="nm")
    mul_eng.tensor_tensor(num[:, :bw], wn[:, :bw], em[:, :bw], op=OP.mult)
    return sp, s2c, den, num, rt


def _emit_f2a(nc, sp, s2c, den, rt):
    """Finish den's second half on GPSIMD (emitted at the next pair's
    first tile, right after the ACT drain that writes s2c's second half,
    and a full half-pair before DVE's rec needs it — no head-of-line
    stall)."""
    nc.gpsimd.tensor_tensor(den[:, rt:2 * rt], sp[:, rt:2 * rt],
                            s2c[:, rt:2 * rt], op=OP.mult)


def _emit_f2(nc, pools, res_sl, sp, s2c, den, num, rt, last):
    """Formula stage 2 (emitted 1.5 tiles later so no engine head-of-line
    stalls on a cross-engine dep): res = num / den."""
    fp_pool = pools[3]
    bw = 2 * rt
    rec = fp_pool.tile([P, BW], f32, tag="rc")
    nc.vector.reciprocal_approx_fast(out=rec[:, :bw], in_=den[:, :bw])
    mul_eng = nc.vector if last else nc.gpsimd
    mul_eng.tensor_tensor(res_sl, num[:, :bw], rec[:, :bw], op=OP.mult)


def _emit_pass(nc, pools, pred, targ, pay_b, ident_b, res, ops, mask_ap):
    _, work_pool, stats_pool, fp_pool, ps_pool, lastp_pool = pools
    embed_op = ops[0]
    pend = None            # F1 outputs of the previous pair block
    pend_drain = None      # (psum_tile, s2c, half, rt) not yet drained
    base = 0               # DRAM row offset of the next tile
    roff = 0               # res column offset of the current pair
    ntiles = 2 * len(PAIR_R)
    ti = 0                 # flat tile counter
    for k, rt in enumerate(PAIR_R):
        bw = 2 * rt
        me = stats_pool.tile([P, BW], f32, tag="me")
        mt = stats_pool.tile([P, BW], f32, tag="mt")
        s2c = fp_pool.tile([P, BW], bf16, tag="s2c")
        # dedicated e2 for the last pair: its exp must not wait for an
        # earlier pair's matmuls to release the shared 2-buf rotation
        if k == len(PAIR_R) - 1:
            e2 = lastp_pool.tile([P, 2 * F], bf16, tag="e2last")
        else:
            e2 = work_pool.tile([P, 2 * F], bf16, tag="e2")
        for sub in range(2):
            nrows = P * rt
            xp_v = pred[base:base + nrows, :].rearrange(
                "(p r) w -> p (r w)", p=P, r=rt)
            xt_v = targ[base:base + nrows, :].rearrange(
                "(p r) w -> p (r w)", p=P, r=rt)
            base += nrows
            _emit_tile(nc, pools, xp_v, xt_v, pay_b, rt, sub, embed_op,
                       mask_ap, me, mt, e2)
            # drain the PREVIOUS tile's PSUM sums now — after this tile's
            # exp in ACT program order, so exp never waits on matmuls
            if pend_drain is not None:
                ps, dst, dhalf, drt = pend_drain
                nc.scalar.activation(dst[:, dhalf * drt:(dhalf + 1) * drt],
                                     ps[:, :drt], ACT.Copy)
                pend_drain = None
            if ti < ntiles - 1:
                pend_drain = (_emit_sums_pe(nc, pools, e2, rt, sub,
                                            ident_b), s2c, sub, rt)
            else:
                _emit_sums_dve_tree(nc, pools, e2, rt, sub, s2c)
            # interleave the previous pair's F2 across this pair
            if pend is not None and sub == 0:
                _emit_f2a(nc, pend[0], pend[1], pend[2], pend[4])
            if pend is not None and sub == 1:
                pbw = 2 * pend[4]
                _emit_f2(nc, pools, res[:, roff - pbw:roff], *pend,
                         last=False)
                pend = None
            ti += 1
        roff += bw
        pend = _emit_f1(nc, pools, me, mt, s2c, rt, ops, mask_ap,
                        last=(k == len(PAIR_R) - 1))
    _emit_f2(nc, pools, res[:, roff - 2 * pend[4]:roff], *pend, last=True)


def _build_program():
    nc = bacc.Bacc("TRN2", target_bir_lowering=False, debug=False)
    pred = nc.dram_tensor("predict", [BS, W], f32, kind="ExternalInput")
    targ = nc.dram_tensor("target", [BS, W], f32, kind="ExternalInput")
    pay = nc.dram_tensor("payload", [P, W], u32, kind="ExternalInput")
    ident = nc.dram_tensor("ident", [P, P], f32, kind="ExternalInput")
    out = nc.dram_tensor("out", [P, RES_COLS], f32, kind="ExternalOutput")

    with tile.TileContext(nc) as tc:
        with (
            tc.tile_pool(name="io", bufs=3) as io_pool,
            tc.tile_pool(name="work", bufs=2) as work_pool,
            tc.tile_pool(name="stats", bufs=2) as stats_pool,
            tc.tile_pool(name="fp", bufs=2) as fp_pool,
            tc.psum_pool(name="ps", bufs=3) as ps_pool,
            tc.tile_pool(name="lastp", bufs=1) as lastp_pool,
            tc.tile_pool(name="const", bufs=1) as const_pool,
        ):
            pay_t = const_pool.tile([P, W], u32, tag="pay")
            nc.gpsimd.dma_start(out=pay_t[:, :], in_=pay[:, :])
            pay_b = pay_t[:, :].unsqueeze(1).broadcast_to([P, R, W]).bitcast(f32)

            ident_t = const_pool.tile([P, P], f32, tag="idf")
            nc.gpsimd.dma_start(out=ident_t[:, :], in_=ident[:, :])
            ident_b = const_pool.tile([P, P], bf16, tag="idb")
            nc.scalar.activation(ident_b[:, :], ident_t[:, :], ACT.Copy)

            mask_t = const_pool.tile([P, 1], u32, tag="mask")
            nc.vector.memset(mask_t[:, :], PAYLOAD_MASK)
            mask_ap = mask_t[:, :1].bitcast(f32)

            res = const_pool.tile([P, RES_COLS], f32, tag="res")

            ops = _register_custom_ops()
            pools = (io_pool, work_pool, stats_pool, fp_pool, ps_pool, lastp_pool)
            _emit_pass(nc, pools, pred[:, :], targ[:, :], pay_b, ident_b,
                       res, ops, mask_ap)

            nc.sync.dma_start(out=out[:, :], in_=res[:, :])
    nc.compile()
    return nc


_CACHE = {}


def _run(predict, target, trace=False):
    if "nc" not in _CACHE:
        _CACHE["nc"] = _build_program()
    nc = _CACHE["nc"]

    predict = np.ascontiguousarray(np.asarray(predict, dtype=np.float32))
    target = np.ascontiguousarray(np.asarray(target, dtype=np.float32))
    payload = np.broadcast_to(
        (np.asarray(LABELS_NUM_COUNT, dtype=np.uint32) // 1000)[None, :], (P, W)
    ).copy()
    ident = np.eye(P, dtype=np.float32)

    in_maps = []
    for i in range(NCORES):
        in_maps.append(
            {
                "predict": predict[i * BS : (i + 1) * BS],
                "target": target[i * BS : (i + 1) * BS],
                "payload": payload,
                "ident": ident,
            }
        )
    res = run_bass_kernel_spmd(nc, in_maps, core_ids=list(range(NCORES)), trace=trace)
    total = np.float64(0.0)
    for r in res.results:
        total += np.float64(r["out"].astype(np.float64).sum())
    value = np.float32(total / B)
    return np.asarray(value, dtype=np.float32), res


def kernel(predict, target, penalty_matrix=None):
    value, _ = _run(predict, target, trace=False)
    return value

